# revision 45
# baseline (speedup 1.0000x reference)
"""Pairwise squared-Euclidean distance map on 8 TRN2 NeuronCores.

d[b, i, j] = sum_c (a[b, c, i] - b[b, c, j])^2
           = aa[b, i] + bb[b, j] - 2 * <a[b, :, i], b[b, :, j]>

Sharding: data-parallel over N (rows of the map). Core k computes
d[:, k*512:(k+1)*512, :] from a[:, :, k*512:(k+1)*512] and the full b.

The PE clock in this environment is pinned at the cold 1.2 GHz (HAM
never unthrottles - measured: 400 dense back-to-back matmuls keep a
427 ns issue gap forever). A single-matmul pipeline therefore floors at
N_cols/1.2GHz = 54.6 us/core. This kernel instead uses 4-way ROW-GROUP
TILING: the 128x128 PE array is addressed as four independent 32-row
groups (tile_position=(32g, 0)), each streaming its own rhs columns
concurrently (measured 130 ns/mm effective for K<=32, N=512 waves).

Each [128, 512] output tile runs as 3 sub-matmuls on ONE row group,
accumulating one PSUM bank:
    K=32  (-2a, channels  0..31) x (b channels  0..31)   start
    K=32  (-2a, channels 32..63) x (b channels 32..63)
    K=2   [ones; aa] x [bb; ones]                        stop
Chunks of 4 tiles (same batch/row-block, jj = 4*half+g) occupy the four
groups concurrently; 8 PSUM banks double-buffer chunks. Effective PE
time: 32 chunks/batch-set * 3 waves * ~0.51 us = ~49 us, just under the
~53 us HBM floor (16 MB fp16 stores + ~3 MB inputs at 358 GB/s), so the
kernel runs DMA-bound.

Norms aa, bb are computed on host (O(B*C*(N+M)) - input-sized prep) and
enter via the K=2 aug matmul; everything heavy stays on device. Inputs
are host-packed into band-major layouts so each 32-partition band holds
exactly the channels/columns its row group streams (no SBUF
replication): band g serves output columns jj in {g, g+4}.

PSUM drains alternate Vector/Scalar engines into [128, 1024] fp16
stages (two tiles per stage so store DMAs move 2 KB per partition row);
stores ride the sync (HWDGE) ring, input DMAs the gpsimd (SWDGE) ring.
"""

import numpy as np
from contextlib import ExitStack

import concourse.bass as bass
import concourse.bacc as bacc
import concourse.mybir as mybir
import concourse.bass_utils as bass_utils
from concourse.tile import TileContext
from concourse.bass_utils import run_bass_kernel_spmd



B, C, N, M = 4, 64, 4096, 4096
NCORES = 8
NSH = N // NCORES          # 512 N rows per core
NB = NSH // 128            # 4 row blocks of 128
MC = 512                   # matmul free dim = one fp32 PSUM bank
NJ = M // MC               # 8 output column tiles

F32 = mybir.dt.float32
F16 = mybir.dt.float16

_CACHE = {}


def _build_nc():
    nc = bacc.Bacc(
        "TRN2",
        target_bir_lowering=False,
        debug=False,
        enable_asserts=True,
        num_devices=NCORES,
    )
    # band-major weights: [bt, g*32+q, i, h, m] = -2*a[bt, 32h+q, i*128+m]
    # (host-replicated per band: SBUF-side replication chains serialize on
    # the ~2us SWDGE latency and stall the PE; 0.75 MB extra HBM is cheaper)
    wa_d = nc.declare_dram_parameter("wa", [B, 128, NB, 2, 128], F16,
                                     isOutput=False)
    # aug rows per band (one DMA per band): cols 0:512 lhsT (row0 ones,
    # row1 aa[bt, i-block]); cols 512:1536 rhs (row0 bb[(g+4s)*512+...],
    # row1 ones)
    aug_d = nc.declare_dram_parameter("aug", [B, 4, 2, 512 + 2 * MC], F16,
                                      isOutput=False)
    # band-major rhs: [bt, g*32+q, h, s, c] = b[bt, 32h+q, (g+4s)*512+c]
    bsh_d = nc.declare_dram_parameter("bsh", [B, 128, 2, 2, MC], F16,
                                      isOutput=False)
    # output cols viewed as (half, pair, 1024) so each drain engine's
    # stage tile stores via one strided DMA with 2KB-contiguous runs
    d_d = nc.declare_dram_parameter("d", [B, NSH, 2, 2, 2 * MC], F16,
                                    isOutput=True)

    with ExitStack() as ctx:
        tc = ctx.enter_context(TileContext(nc))
        wap = ctx.enter_context(tc.tile_pool(name="wa", bufs=2))
        augp = ctx.enter_context(tc.tile_pool(name="aug", bufs=2))
        bshp = ctx.enter_context(tc.tile_pool(name="bsh", bufs=2))
        mpsum = ctx.enter_context(tc.tile_pool(name="mp", bufs=2, space="PSUM"))
        stage = ctx.enter_context(tc.tile_pool(name="st", bufs=3))

        # Drain assignment: Vector always drains psum pair 0, Scalar pair 1,
        # each into its OWN stage tile (a shared tile serializes the two
        # engines via WAW ordering), so both pairs release in parallel
        # ~1.2us after the aug wave.

        def prefetch(bt, eng, eng2=None):
            """Issue input DMAs in first-use order. For bt=0 the two
            wave-0-critical DMAs ride the sync (HWDGE) ring - stores
            haven't started yet and each trigger costs ~0.6 us of engine
            time - while the rest go to the gpsimd (SWDGE) ring, which is
            also free early. Later batches use gpsimd only."""
            eng2 = eng2 or eng
            wa = wap.tile([128, NB, 2, 128], F16, tag="wa", name=f"wa{bt}")
            bsh = bshp.tile([128, 2, 2, MC], F16, tag="bsh", name=f"bsh{bt}")
            aug = augp.tile([128, 512 + 2 * MC], F16, tag="aug",
                            name=f"au{bt}")
            # rows between the per-band aug pairs are never DMA'd but get
            # swept up by the wide aug LDWEIGHTS - zero them once
            nc.gpsimd.memset(aug[:, 0:512], 0)
            eng.dma_start(out=wa[:, 0, :, :], in_=wa_d[bt, :, 0])
            eng.dma_start(out=bsh[:, :, 0, :], in_=bsh_d[bt, :, :, 0])
            for g in range(4):
                eng2.dma_start(out=aug[32 * g:32 * g + 2, :],
                               in_=aug_d[bt, g])
            eng2.dma_start(out=bsh[:, :, 1, :], in_=bsh_d[bt, :, :, 1])
            eng2.dma_start(out=wa[:, 1:NB, :, :], in_=wa_d[bt, :, 1:NB])
            return wa, aug, bsh

        def chunk(bt, i, half, tiles, sags):
            wa, aug, bsh = tiles
            # two [128,1024] psum tiles (2 banks each); group g writes the
            # (g%2) half of tile g//2 -> one drain per pair
            pts = [
                mpsum.tile([128, 2 * MC], F32, tag=f"p{p}",
                           name=f"p{bt}_{i}_{half}_{p}")
                for p in range(2)
            ]

            def pout(g):
                return pts[g // 2][:, (g % 2) * MC:(g % 2 + 1) * MC]

            # One wide [128,128] LDWEIGHTS per wave loads all 4 row groups
            # (LDW time is column-bound: 32-row and 128-row loads both take
            # ~107ns of weight-port time, and 4 narrow loads per 427ns wave
            # oversubscribe the port). The wave's matmuls are marked
            # non-self-loading.
            for h in range(2):  # two K=32 data waves
                nc.tensor.ldweights(weights=wa[:, i, h, :])
                for g in range(4):
                    mi = nc.tensor.matmul(
                        pout(g),
                        wa[32 * g:32 * g + 32, i, h, :],
                        bsh[32 * g:32 * g + 32, h, half, :],
                        start=(h == 0), stop=False, skip_group_check=True,
                        tile_position=(32 * g, 0),
                    )
                    mi.ins.ldweights = False
            nc.tensor.ldweights(weights=aug[:, i * 128:(i + 1) * 128])
            for g in range(4):  # aug wave: + aa[i] + bb[jj cols]
                mi = nc.tensor.matmul(
                    pout(g),
                    aug[32 * g:32 * g + 2, i * 128:(i + 1) * 128],
                    aug[32 * g:32 * g + 2, 512 + half * MC:512 + (half + 1) * MC],
                    start=False, stop=True, skip_group_check=True,
                    tile_position=(32 * g, 0),
                )
                mi.ins.ldweights = False
            stv, sts = sags
            nc.vector.tensor_copy(
                stv[:, half * 2 * MC:(half + 1) * 2 * MC], pts[0][:, :]
            )
            nc.scalar.copy(
                sts[:, half * 2 * MC:(half + 1) * 2 * MC], pts[1][:, :]
            )
            if half == 1:  # both halves drained -> 2 strided 512KB stores
                nc.sync.dma_start(
                    out=d_d[bt, i * 128:(i + 1) * 128, :, 0, :],
                    in_=stv[:, :],
                )
                nc.sync.dma_start(
                    out=d_d[bt, i * 128:(i + 1) * 128, :, 1, :],
                    in_=sts[:, :],
                )

        tiles = prefetch(0, nc.sync, nc.gpsimd)
        for bt in range(B):
            for i in range(NB):
                sags = (
                    stage.tile([128, 4 * MC], F16, tag="sv",
                               name=f"sv{bt}_{i}"),
                    stage.tile([128, 4 * MC], F16, tag="ss",
                               name=f"ss{bt}_{i}"),
                )
                for half in range(2):
                    chunk(bt, i, half, tiles, sags)
                    if bt + 1 < B and i == 0 and half == 1:
                        ntiles = prefetch(bt + 1, nc.gpsimd)
            if bt + 1 < B:
                tiles = ntiles

    nc.compile()
    return nc


def _get_nc():
    if "nc" not in _CACHE:
        _CACHE["nc"] = _build_nc()
    return _CACHE["nc"]


def _make_in_maps(a, b):
    a = np.asarray(a, dtype=np.float32)
    b = np.asarray(b, dtype=np.float32)
    aa = np.einsum("bcn,bcn->bn", a, a)            # [B, N] fp32
    bb = np.einsum("bcm,bcm->bm", b, b)            # [B, M] fp32
    b16 = b.astype(np.float16)
    na16 = (-2.0 * a).astype(np.float16)

    # band-major rhs: bsh[bt, g, q, h, s, c] = b[bt, 32h+q, (g+4s)*512+c]
    bsh = np.empty((B, 4, 32, 2, 2, MC), dtype=np.float16)
    for g in range(4):
        for s in range(2):
            jj = g + 4 * s
            seg = b16[:, :, jj * MC:(jj + 1) * MC].reshape(B, 2, 32, MC)
            bsh[:, g, :, :, s, :] = seg.transpose(0, 2, 1, 3)
    bsh = bsh.reshape(B, 128, 2 * 2 * MC)

    in_maps = []
    for k in range(NCORES):
        asl = na16[:, :, k * NSH:(k + 1) * NSH]    # [B, C, 512]
        # wa[bt, g, q, i, h, m] = -2a[bt, 32h+q, i*128+m] (same per band g)
        wa1 = asl.reshape(B, 2, 32, NB, 128).transpose(0, 2, 3, 1, 4)
        wa = np.broadcast_to(
            wa1[:, None], (B, 4, 32, NB, 2, 128)
        ).reshape(B, 128, NB, 2, 128)
        aasl = aa[:, k * NSH:(k + 1) * NSH].astype(np.float16)  # [B, 512]
        # merged aug rows: cols 0:512 lhsT [ones; aa], 512:1536 rhs [bb; 1]
        aug = np.empty((B, 4, 2, 512 + 2 * MC), dtype=np.float16)
        aug[:, :, 0, 0:512] = 1.0
        aug[:, :, 1, 0:512] = aasl[:, None, :]
        for g in range(4):
            for s in range(2):
                jj = g + 4 * s
                aug[:, g, 0, 512 + s * MC:512 + (s + 1) * MC] = \
                    bb[:, jj * MC:(jj + 1) * MC]
        aug[:, :, 1, 512:] = 1.0
        in_maps.append(
            {
                "wa": np.ascontiguousarray(wa),
                "aug": aug,
                "bsh": bsh,
            }
        )
    return in_maps


def kernel(a, b, _trace=False, _trace_kwargs=None):
    nc = _get_nc()
    in_maps = _make_in_maps(a, b)
    res = run_bass_kernel_spmd(
        nc,
        in_maps,
        core_ids=list(range(NCORES)),
        trace=_trace,
        **(_trace_kwargs or {}),
    )
    out = np.concatenate(
        [res.results[k]["d"].reshape(B, NSH, M) for k in range(NCORES)],
        axis=1,
    ).astype(np.float32)
    if _trace:
        _CACHE["last_results"] = res
    return out
